# revision 15
# baseline (speedup 1.0000x reference)
"""FFM layer (embedding lookup + field-factorization) on 8 trn2 NeuronCores.

Strategy: data-parallel over batch (4096 rows -> 512/core), embedding tables
replicated to every core.  The reference's inner reduction
  latent_sum[b,f,k] = sum_j v[idx[b,f], j, k]
sums over ALL 26 fields j regardless of the batch indices, so
  vred[i,k] = sum_j v[i,j,k]
is a pure function of the parameters and is folded into the table host-side
(same spirit as packing w into the augmented table).  Likewise the
second-order self term and first-order weight fold into one row scalar
  c[i] = w[i] + w0/26 - 0.5*|vred[i]|^2,
leaving the device with
  out[b] = sum_f c[idx[b,f]] + 0.5 * |sum_f vred[idx[b,f]]|^2.

Each table row is [vred (8 f32) | c | pad] = 64 f32 = 256 B, the SWDGE
minimum elem size.  Lookups use the SWDGE dma_gather custom instruction,
one per field (field-local int16 indices, 512 per gather), spread over all
4 SWDGE queues.  Hardware-measured wall: each queue's descriptor FIFO
sustains ~8-9 ns/descriptor regardless of row size, packet aggregation
(single_packet True or False) or SDMA-engine spread, so the 13312
descriptors take ~30-34us on 4 queues.  Every alternative gather path was
HW-probed and is slower: the Pool indirect1d DMA is a single FIFO, its
descriptor stream corrupts beyond the first `A` chunks of a [p, A, B]
dest AP (~10% survive), and its reliable 2-D form moves only 128 rows
per ~1.1us instruction; ap_gather-style SBUF-resident tables cost more in
table broadcast DMA than they save; HWDGE only does affine patterns.

Changes vs the first working version (66.5 -> 65.2us):
  - gpsimd.load_library(mlp) hoisted to the top of the Pool stream so the
    gather ucode's ~6us Q7 IRAM load overlaps the idx upload instead of
    sitting in front of the first gather (do NOT touch gpsimd.memset
    etc. before the gathers: each extra Pool library costs its own IRAM
    load -- hardware-measured +11us);
  - idx upload via the Sync engine's HWDGE (was SWDGE queue 0), split in
    two so the first fields' gathers are never idx-gated;
  - one hoisted num_idxs register instead of 26 MOVEs;
  - index ordinal i = batch row lands at dest [i % 128, i // 128, :],
    the (partition, batch-tile) layout the VectorE tail wants; a 22/4
    field split hides the big reduce under the last gathers, and the two
    deepest queues (7 gathers) hold the late-reduced fields 24/25.
"""

import sys

import numpy as np

FIELD = 26
K = 8
RPAD = 64                # padded row length in f32 (256 B, SWDGE minimum)
VOCAB = 20000
TOTAL = FIELD * VOCAB    # 520000
B = 4096
NCORES = 8
BC = B // NCORES         # 512 batch rows per core
P = 128
NTILES = BC // P         # 4
NSLOT = BC // 16         # 32 int16 index slots per idx partition
FSPLIT = 8               # fields in the first (early) idx upload

_TRN_REPO = "/opt/trn_rl_repo"

_cache = {}


def _build_nc(n_iters=1):
    if _TRN_REPO not in sys.path:
        sys.path.insert(0, _TRN_REPO)
    from concourse import bacc, library_config, mybir, tile

    f32 = mybir.dt.float32
    i16 = mybir.dt.int16
    Alu = mybir.AluOpType
    Ax = mybir.AxisListType

    nc = bacc.Bacc("TRN2", target_bir_lowering=False, debug=False,
                   num_swdge_queues=4)
    # idx16[p, f, s] = int16 field-local index of batch row s*16+(p%16),
    # field f -- 16-partition wrap replicated to 128 host-side
    idx_d = nc.dram_tensor("idx16", [P, FIELD, NSLOT], i16,
                           kind="ExternalInput")
    tab_d = nc.dram_tensor("tab", [TOTAL, RPAD], f32, kind="ExternalInput")
    out_d = nc.dram_tensor("out", [BC, 1], f32, kind="ExternalOutput")

    # -- hoist the dma_gather ucode's ~6us Q7 IRAM load as early as the
    # Pool stream allows (before the TileContext's own preamble ops) so it
    # overlaps the engine preamble / idx upload; otherwise the
    # auto-inserted MODIFY_POOL_CONFIG sits right before the first gather
    # and delays it to ~16-18us
    nc.gpsimd.load_library(library_config.mlp)

    with tile.TileContext(nc) as tc:
        with tc.tile_pool(name="pool", bufs=1) as pool:
            for _ in range(n_iters):

                # -- idx upload on the Sync engine's HWDGE, split so the
                # first FSPLIT fields' gathers can start sooner
                idx_sb = pool.tile([P, FIELD, NSLOT], i16, tag="idx")
                nc.sync.dma_start(out=idx_sb[:, 0:FSPLIT, :],
                                  in_=idx_d[:, 0:FSPLIT, :])
                nc.sync.dma_start(out=idx_sb[:, FSPLIT:, :],
                                  in_=idx_d[:, FSPLIT:, :])

                # vg[p, f, t, :] = tab[f*VOCAB + idx[t*128+p, f], :]
                # one dma_gather per field, round-robin over the 4 SWDGE
                # queues; each queue FIFO sustains ~9ns/desc so the 26
                # gathers' 13312 descriptors drain in ~28-30us total
                HF = 22  # 22/4 split: the 22-field reduce hides under the
                #          last gathers; only a 4-field reduce is exposed
                nidx_reg = nc.gpsimd.to_reg(BC)  # one MOVE, not 26
                vgA = pool.tile([P, HF, NTILES, RPAD], f32, tag="vga")
                vgB = pool.tile([P, FIELD - HF, NTILES, RPAD], f32,
                                tag="vgb")
                nidx_half = nc.gpsimd.to_reg(BC // 2)
                for f in range(24):
                    vgh, fh = (vgA, f) if f < HF else (vgB, f - HF)
                    nc.gpsimd.dma_gather(
                        out_ap=vgh[:, fh],
                        in_ap=tab_d[f * VOCAB:(f + 1) * VOCAB, :],
                        idxs_ap=idx_sb[:, f, :],
                        num_idxs=BC,
                        num_idxs_reg=nidx_reg,
                        elem_size=RPAD,
                        single_packet=False,
                        queue_num=f % 4,
                    )
                # fields 24/25 are the 7th (last) round: split each into
                # two 256-idx half-gathers so the final round is ~2.5us
                # deep on all 4 queues instead of ~4.9us on two of them
                for i, (f, half) in enumerate(
                        ((24, 0), (24, 1), (25, 0), (25, 1))):
                    nc.gpsimd.dma_gather(
                        out_ap=vgB[:, f - HF, 2 * half:2 * half + 2],
                        in_ap=tab_d[f * VOCAB:(f + 1) * VOCAB, :],
                        idxs_ap=idx_sb[:, f, 16 * half:16 * half + 16],
                        num_idxs=BC // 2,
                        num_idxs_reg=nidx_half,
                        elem_size=RPAD,
                        single_packet=False,
                        queue_num=i,
                    )

                # per-half partial reductions; half A overlaps half B's
                # gathers on the Pool engine
                sA = pool.tile([P, NTILES, K], f32, tag="sA")
                cA = pool.tile([P, NTILES], f32, tag="cA")
                sB = pool.tile([P, NTILES, K], f32, tag="sB")
                cB = pool.tile([P, NTILES], f32, tag="cB")
                for vgh, sh, ch in ((vgA, sA, cA), (vgB, sB, cB)):
                    nc.vector.tensor_reduce(
                        out=sh[:],
                        in_=vgh[:, :, :, 0:K].rearrange(
                            "p f t k -> p t k f"),
                        axis=Ax.X,
                        op=Alu.add,
                    )
                    nc.vector.tensor_reduce(
                        out=ch[:],
                        in_=vgh[:, :, :, K].rearrange("p f t -> p t f"),
                        axis=Ax.X,
                        op=Alu.add,
                    )
                s_all = pool.tile([P, NTILES, K], f32, tag="s")
                nc.vector.tensor_tensor(
                    out=s_all[:], in0=sA[:], in1=sB[:], op=Alu.add
                )
                csum = pool.tile([P, NTILES], f32, tag="c")
                nc.vector.tensor_tensor(
                    out=csum[:], in0=cA[:], in1=cB[:], op=Alu.add
                )
                # table stores vred*sqrt(0.5), so |s|^2 here IS the
                # 0.5*|sum vred|^2 term -- no scalar-mul needed
                ssq = pool.tile([P, NTILES, K], f32, tag="ssq")
                nc.vector.tensor_tensor(
                    out=ssq[:], in0=s_all[:], in1=s_all[:], op=Alu.mult
                )
                s2 = pool.tile([P, NTILES], f32, tag="s2")
                nc.vector.tensor_reduce(
                    out=s2[:], in_=ssq[:], axis=Ax.X, op=Alu.add
                )
                out_all = pool.tile([P, NTILES], f32, tag="oa")
                nc.vector.tensor_tensor(
                    out=out_all[:], in0=s2[:], in1=csum[:], op=Alu.add
                )
                # single store: out[t*128+p] = out_all[p, t]
                nc.sync.dma_start(
                    out=out_d[:, :].rearrange("(t p) one -> p (t one)", p=P),
                    in_=out_all[:],
                )
    nc.compile()
    return nc


def get_nc():
    if "nc" not in _cache:
        _cache["nc"] = _build_nc()
    return _cache["nc"]


def make_in_maps(inputs, offsets, w0, w, v):
    del offsets  # folded into the per-field subtable slicing
    inp = np.asarray(inputs)
    idx16 = np.ascontiguousarray(
        inp.astype(np.int16).reshape(NCORES, BC, FIELD)
    )
    # reduced table row: [vred (8 f32) | c | pad to 64 f32 = 256 B]
    vred = np.asarray(v, dtype=np.float32).reshape(TOTAL, FIELD, K).sum(axis=1)
    c = (np.asarray(w, dtype=np.float32).reshape(TOTAL)
         + np.float32(np.asarray(w0, np.float32).reshape(()) / FIELD)
         - 0.5 * (vred * vred).sum(axis=1))
    tab = np.zeros((TOTAL, RPAD), dtype=np.float32)
    # sqrt(0.5) fold: the device's |sum_f row|^2 then equals 0.5*|s|^2
    tab[:, :K] = vred * np.float32(np.sqrt(0.5))
    tab[:, K] = c
    maps = []
    for i in range(NCORES):
        shard = idx16[i]                       # [BC, FIELD]
        wrapped = shard.reshape(NSLOT, 16, FIELD).transpose(1, 2, 0)
        # [16, FIELD, NSLOT] -> replicate to 128 partitions
        rep = np.ascontiguousarray(np.tile(wrapped, (NCORES, 1, 1)))
        maps.append({"idx16": rep, "tab": tab})
    return maps


def assemble_out(res):
    return np.concatenate(
        [np.asarray(res.results[i]["out"]) for i in range(NCORES)], axis=0
    ).astype(np.float32)


def kernel(inputs, offsets, w0, w, v):
    if _TRN_REPO not in sys.path:
        sys.path.insert(0, _TRN_REPO)
    from concourse.bass_utils import run_bass_kernel_spmd

    nc = get_nc()
    in_maps = make_in_maps(inputs, offsets, w0, w, v)
    res = run_bass_kernel_spmd(nc, in_maps, list(range(NCORES)))
    return assemble_out(res)


# revision 17
# speedup vs baseline: 1.1332x; 1.1332x over previous
"""FFM layer (embedding lookup + field-factorization) on 8 trn2 NeuronCores.

Strategy: data-parallel over batch (4096 rows -> 512/core), embedding tables
replicated to every core.  The reference's inner reduction
  latent_sum[b,f,k] = sum_j v[idx[b,f], j, k]
sums over ALL 26 fields j regardless of the batch indices, so
  vred[i,k] = sum_j v[i,j,k]
is a pure function of the parameters and is folded into the table host-side
(same spirit as packing w into the augmented table).  Likewise the
second-order self term and first-order weight fold into one row scalar
  c[i] = w[i] + w0/26 - 0.5*|vred[i]|^2,
leaving the device with
  out[b] = sum_f c[idx[b,f]] + 0.5 * |sum_f vred[idx[b,f]]|^2.

Each table row is [vred (8 f32) | c | pad] = 64 f32 = 256 B, the SWDGE
minimum elem size.  Lookups use the SWDGE dma_gather custom instruction,
one per field (field-local int16 indices, 512 per gather), spread over all
4 SWDGE queues.  Hardware-measured wall: each queue's descriptor FIFO
sustains ~8-9 ns/descriptor regardless of row size, packet aggregation
(single_packet True or False) or SDMA-engine spread, so the 13312
descriptors take ~30-34us on 4 queues.  Every alternative gather path was
HW-probed and is slower: the Pool indirect1d DMA is a single FIFO, its
descriptor stream corrupts beyond the first `A` chunks of a [p, A, B]
dest AP (~10% survive), and its reliable 2-D form moves only 128 rows
per ~1.1us instruction; ap_gather-style SBUF-resident tables cost more in
table broadcast DMA than they save; HWDGE only does affine patterns.

Changes vs the first working version (66.5 -> 65.2us):
  - gpsimd.load_library(mlp) hoisted to the top of the Pool stream so the
    gather ucode's ~6us Q7 IRAM load overlaps the idx upload instead of
    sitting in front of the first gather (do NOT touch gpsimd.memset
    etc. before the gathers: each extra Pool library costs its own IRAM
    load -- hardware-measured +11us);
  - idx upload via the Sync engine's HWDGE (was SWDGE queue 0), split in
    two so the first fields' gathers are never idx-gated;
  - one hoisted num_idxs register instead of 26 MOVEs;
  - index ordinal i = batch row lands at dest [i % 128, i // 128, :],
    the (partition, batch-tile) layout the VectorE tail wants; a 22/4
    field split hides the big reduce under the last gathers, and the two
    deepest queues (7 gathers) hold the late-reduced fields 24/25.
"""

import sys

import numpy as np

FIELD = 26
K = 8
RPAD = 64                # padded row length in f32 (256 B, SWDGE minimum)
VOCAB = 20000
TOTAL = FIELD * VOCAB    # 520000
B = 4096
NCORES = 8
BC = B // NCORES         # 512 batch rows per core
P = 128
NTILES = BC // P         # 4
NSLOT = BC // 16         # 32 int16 index slots per idx partition
FSPLIT = 8               # fields in the first (early) idx upload

_TRN_REPO = "/opt/trn_rl_repo"

_cache = {}


def _build_nc(n_iters=1):
    if _TRN_REPO not in sys.path:
        sys.path.insert(0, _TRN_REPO)
    from concourse import bacc, library_config, mybir, tile

    f32 = mybir.dt.float32
    i16 = mybir.dt.int16
    Alu = mybir.AluOpType
    Ax = mybir.AxisListType

    nc = bacc.Bacc("TRN2", target_bir_lowering=False, debug=False,
                   num_swdge_queues=4)
    # idx16[p, f, s] = int16 field-local index of batch row s*16+(p%16),
    # field f -- 16-partition wrap replicated to 128 host-side
    idx_d = nc.dram_tensor("idx16", [P, FIELD, NSLOT], i16,
                           kind="ExternalInput")
    tab_d = nc.dram_tensor("tab", [TOTAL, RPAD], f32, kind="ExternalInput")
    out_d = nc.dram_tensor("out", [BC, 1], f32, kind="ExternalOutput")

    with tile.TileContext(nc) as tc:
        with tc.tile_pool(name="pool", bufs=1) as pool:
            for _ in range(n_iters):
                # -- hoist the dma_gather ucode's ~6us Q7 IRAM load to the
                # top of the Pool stream so it overlaps the idx upload;
                # otherwise the auto-inserted MODIFY_POOL_CONFIG sits right
                # before the first gather and delays it to ~16-18us
                nc.gpsimd.load_library(library_config.mlp)

                # -- idx upload on the Sync engine's HWDGE, split so the
                # first FSPLIT fields' gathers can start sooner
                idx_sb = pool.tile([P, FIELD, NSLOT], i16, tag="idx")
                nc.sync.dma_start(out=idx_sb[:, 0:FSPLIT, :],
                                  in_=idx_d[:, 0:FSPLIT, :])
                nc.sync.dma_start(out=idx_sb[:, FSPLIT:, :],
                                  in_=idx_d[:, FSPLIT:, :])

                # vg[p, f, t, :] = tab[f*VOCAB + idx[t*128+p, f], :]
                # one dma_gather per field, round-robin over the 4 SWDGE
                # queues; each queue FIFO sustains ~9ns/desc so the 26
                # gathers' 13312 descriptors drain in ~28-30us total
                HF = 22  # 22/4 split: the 22-field reduce hides under the
                #          last gathers; only a 4-field reduce is exposed
                nidx_reg = nc.gpsimd.to_reg(BC)  # one MOVE, not 26
                vgA = pool.tile([P, HF, NTILES, RPAD], f32, tag="vga")
                vgB = pool.tile([P, FIELD - HF, NTILES, RPAD], f32,
                                tag="vgb")
                for f in range(FIELD):
                    vgh, fh = (vgA, f) if f < HF else (vgB, f - HF)
                    nc.gpsimd.dma_gather(
                        out_ap=vgh[:, fh],
                        in_ap=tab_d[f * VOCAB:(f + 1) * VOCAB, :],
                        idxs_ap=idx_sb[:, f, :],
                        num_idxs=BC,
                        num_idxs_reg=nidx_reg,
                        elem_size=RPAD,
                        single_packet=False,
                        queue_num=f % 4,
                    )

                # per-half partial reductions; half A overlaps half B's
                # gathers on the Pool engine
                sA = pool.tile([P, NTILES, K], f32, tag="sA")
                cA = pool.tile([P, NTILES], f32, tag="cA")
                sB = pool.tile([P, NTILES, K], f32, tag="sB")
                cB = pool.tile([P, NTILES], f32, tag="cB")
                for vgh, sh, ch in ((vgA, sA, cA), (vgB, sB, cB)):
                    nc.vector.tensor_reduce(
                        out=sh[:],
                        in_=vgh[:, :, :, 0:K].rearrange(
                            "p f t k -> p t k f"),
                        axis=Ax.X,
                        op=Alu.add,
                    )
                    nc.vector.tensor_reduce(
                        out=ch[:],
                        in_=vgh[:, :, :, K].rearrange("p f t -> p t f"),
                        axis=Ax.X,
                        op=Alu.add,
                    )
                s_all = pool.tile([P, NTILES, K], f32, tag="s")
                nc.vector.tensor_tensor(
                    out=s_all[:], in0=sA[:], in1=sB[:], op=Alu.add
                )
                csum = pool.tile([P, NTILES], f32, tag="c")
                nc.vector.tensor_tensor(
                    out=csum[:], in0=cA[:], in1=cB[:], op=Alu.add
                )
                # table stores vred*sqrt(0.5), so |s|^2 here IS the
                # 0.5*|sum vred|^2 term -- no scalar-mul needed
                ssq = pool.tile([P, NTILES, K], f32, tag="ssq")
                nc.vector.tensor_tensor(
                    out=ssq[:], in0=s_all[:], in1=s_all[:], op=Alu.mult
                )
                s2 = pool.tile([P, NTILES], f32, tag="s2")
                nc.vector.tensor_reduce(
                    out=s2[:], in_=ssq[:], axis=Ax.X, op=Alu.add
                )
                out_all = pool.tile([P, NTILES], f32, tag="oa")
                nc.vector.tensor_tensor(
                    out=out_all[:], in0=s2[:], in1=csum[:], op=Alu.add
                )
                # single store: out[t*128+p] = out_all[p, t]
                nc.sync.dma_start(
                    out=out_d[:, :].rearrange("(t p) one -> p (t one)", p=P),
                    in_=out_all[:],
                )
    nc.compile()
    return nc


def get_nc():
    if "nc" not in _cache:
        _cache["nc"] = _build_nc()
    return _cache["nc"]


def make_in_maps(inputs, offsets, w0, w, v):
    del offsets  # folded into the per-field subtable slicing
    inp = np.asarray(inputs)
    idx16 = np.ascontiguousarray(
        inp.astype(np.int16).reshape(NCORES, BC, FIELD)
    )
    # reduced table row: [vred (8 f32) | c | pad to 64 f32 = 256 B]
    vred = np.asarray(v, dtype=np.float32).reshape(TOTAL, FIELD, K).sum(axis=1)
    c = (np.asarray(w, dtype=np.float32).reshape(TOTAL)
         + np.float32(np.asarray(w0, np.float32).reshape(()) / FIELD)
         - 0.5 * (vred * vred).sum(axis=1))
    tab = np.zeros((TOTAL, RPAD), dtype=np.float32)
    # sqrt(0.5) fold: the device's |sum_f row|^2 then equals 0.5*|s|^2
    tab[:, :K] = vred * np.float32(np.sqrt(0.5))
    tab[:, K] = c
    maps = []
    for i in range(NCORES):
        shard = idx16[i]                       # [BC, FIELD]
        wrapped = shard.reshape(NSLOT, 16, FIELD).transpose(1, 2, 0)
        # [16, FIELD, NSLOT] -> replicate to 128 partitions
        rep = np.ascontiguousarray(np.tile(wrapped, (NCORES, 1, 1)))
        maps.append({"idx16": rep, "tab": tab})
    return maps


def assemble_out(res):
    return np.concatenate(
        [np.asarray(res.results[i]["out"]) for i in range(NCORES)], axis=0
    ).astype(np.float32)


def kernel(inputs, offsets, w0, w, v):
    if _TRN_REPO not in sys.path:
        sys.path.insert(0, _TRN_REPO)
    from concourse.bass_utils import run_bass_kernel_spmd

    nc = get_nc()
    in_maps = make_in_maps(inputs, offsets, w0, w, v)
    res = run_bass_kernel_spmd(nc, in_maps, list(range(NCORES)))
    return assemble_out(res)


# revision 18
# speedup vs baseline: 1.1637x; 1.0269x over previous
"""FFM layer (embedding lookup + field-factorization) on 8 trn2 NeuronCores.

Strategy: data-parallel over batch (4096 rows -> 512/core), embedding tables
replicated to every core.  The reference's inner reduction
  latent_sum[b,f,k] = sum_j v[idx[b,f], j, k]
sums over ALL 26 fields j regardless of the batch indices, so
  vred[i,k] = sum_j v[i,j,k]
is a pure function of the parameters and is folded into the table host-side
(same spirit as packing w into the augmented table).  Likewise the
second-order self term and first-order weight fold into one row scalar
  c[i] = w[i] + w0/26 - 0.5*|vred[i]|^2,
leaving the device with
  out[b] = sum_f c[idx[b,f]] + 0.5 * |sum_f vred[idx[b,f]]|^2.

Each table row is [vred (8 f32) | c | pad] = 64 f32 = 256 B, the SWDGE
minimum elem size.  Lookups use the SWDGE dma_gather custom instruction,
one per field (field-local int16 indices, 512 per gather), spread over all
4 SWDGE queues.  Hardware-measured wall: each queue's descriptor FIFO
sustains ~8-9 ns/descriptor regardless of row size, packet aggregation
(single_packet True or False) or SDMA-engine spread, so the 13312
descriptors take ~30-34us on 4 queues.  Every alternative gather path was
HW-probed and is slower: the Pool indirect1d DMA is a single FIFO, its
descriptor stream corrupts beyond the first `A` chunks of a [p, A, B]
dest AP (~10% survive), and its reliable 2-D form moves only 128 rows
per ~1.1us instruction; ap_gather-style SBUF-resident tables cost more in
table broadcast DMA than they save; HWDGE only does affine patterns.

Changes vs the first working version (66.5 -> 65.2us):
  - gpsimd.load_library(mlp) hoisted to the top of the Pool stream so the
    gather ucode's ~6us Q7 IRAM load overlaps the idx upload instead of
    sitting in front of the first gather (do NOT touch gpsimd.memset
    etc. before the gathers: each extra Pool library costs its own IRAM
    load -- hardware-measured +11us);
  - idx upload via the Sync engine's HWDGE (was SWDGE queue 0), split in
    two so the first fields' gathers are never idx-gated;
  - one hoisted num_idxs register instead of 26 MOVEs;
  - index ordinal i = batch row lands at dest [i % 128, i // 128, :],
    the (partition, batch-tile) layout the VectorE tail wants; a 22/4
    field split hides the big reduce under the last gathers, and the two
    deepest queues (7 gathers) hold the late-reduced fields 24/25.
"""

import sys

import numpy as np

FIELD = 26
K = 8
RPAD = 64                # padded row length in f32 (256 B, SWDGE minimum)
VOCAB = 20000
TOTAL = FIELD * VOCAB    # 520000
B = 4096
NCORES = 8
BC = B // NCORES         # 512 batch rows per core
P = 128
NTILES = BC // P         # 4
NSLOT = BC // 16         # 32 int16 index slots per idx partition
FSPLIT = 8               # fields in the first (early) idx upload

_TRN_REPO = "/opt/trn_rl_repo"

_cache = {}


def _build_nc(n_iters=1):
    if _TRN_REPO not in sys.path:
        sys.path.insert(0, _TRN_REPO)
    from concourse import bacc, library_config, mybir, tile

    f32 = mybir.dt.float32
    i16 = mybir.dt.int16
    Alu = mybir.AluOpType
    Ax = mybir.AxisListType

    nc = bacc.Bacc("TRN2", target_bir_lowering=False, debug=False,
                   num_swdge_queues=4)
    # idx16[p, f, s] = int16 field-local index of batch row s*16+(p%16),
    # field f -- 16-partition wrap replicated to 128 host-side
    idx_d = nc.dram_tensor("idx16", [P, FIELD, NSLOT], i16,
                           kind="ExternalInput")
    tab_d = nc.dram_tensor("tab", [TOTAL, RPAD], f32, kind="ExternalInput")
    out_d = nc.dram_tensor("out", [BC, 1], f32, kind="ExternalOutput")

    with tile.TileContext(nc) as tc:
        with tc.tile_pool(name="pool", bufs=1) as pool:
            for _ in range(n_iters):
                # -- hoist the dma_gather ucode's ~6us Q7 IRAM load to the
                # top of the Pool stream so it overlaps the idx upload;
                # otherwise the auto-inserted MODIFY_POOL_CONFIG sits right
                # before the first gather and delays it to ~16-18us
                nc.gpsimd.load_library(library_config.mlp)

                # -- idx upload on the Sync engine's HWDGE, split so the
                # first FSPLIT fields' gathers can start sooner
                idx_sb = pool.tile([P, FIELD, NSLOT], i16, tag="idx")
                nc.sync.dma_start(out=idx_sb[:, 0:FSPLIT, :],
                                  in_=idx_d[:, 0:FSPLIT, :])
                nc.sync.dma_start(out=idx_sb[:, FSPLIT:, :],
                                  in_=idx_d[:, FSPLIT:, :])

                # vg[p, f, t, :] = tab[f*VOCAB + idx[t*128+p, f], :]
                # one dma_gather per field, round-robin over the 4 SWDGE
                # queues; each queue FIFO sustains ~9ns/desc so the 26
                # gathers' 13312 descriptors drain in ~28-30us total
                HF = 22  # 22/4 split: the 22-field reduce hides under the
                #          last gathers; only a 4-field reduce is exposed
                nidx_reg = nc.gpsimd.to_reg(BC)  # one MOVE, not 26
                vgA = pool.tile([P, HF, NTILES, RPAD], f32, tag="vga")
                vgB = pool.tile([P, FIELD - HF, NTILES, RPAD], f32,
                                tag="vgb")
                for f in range(24):
                    vgh, fh = (vgA, f) if f < HF else (vgB, f - HF)
                    nc.gpsimd.dma_gather(
                        out_ap=vgh[:, fh],
                        in_ap=tab_d[f * VOCAB:(f + 1) * VOCAB, :],
                        idxs_ap=idx_sb[:, f, :],
                        num_idxs=BC,
                        num_idxs_reg=nidx_reg,
                        elem_size=RPAD,
                        single_packet=False,
                        queue_num=f % 4,
                    )
                # fields 24/25 split into 256-idx half-gathers, one per
                # queue: every queue then carries exactly 3328 descriptors
                # (6.5 x 512) instead of 3584/3584/3072/3072, shaving the
                # two deepest queues' extra half-round off the drain wall
                nidx_half = nc.gpsimd.to_reg(BC // 2)
                for i, (f, half) in enumerate(
                        ((24, 0), (24, 1), (25, 0), (25, 1))):
                    nc.gpsimd.dma_gather(
                        out_ap=vgB[:, f - HF, 2 * half:2 * half + 2],
                        in_ap=tab_d[f * VOCAB:(f + 1) * VOCAB, :],
                        idxs_ap=idx_sb[:, f, 16 * half:16 * half + 16],
                        num_idxs=BC // 2,
                        num_idxs_reg=nidx_half,
                        elem_size=RPAD,
                        single_packet=False,
                        queue_num=i,
                    )

                # per-half partial reductions; half A overlaps half B's
                # gathers on the Pool engine
                sA = pool.tile([P, NTILES, K], f32, tag="sA")
                cA = pool.tile([P, NTILES], f32, tag="cA")
                sB = pool.tile([P, NTILES, K], f32, tag="sB")
                cB = pool.tile([P, NTILES], f32, tag="cB")
                for vgh, sh, ch in ((vgA, sA, cA), (vgB, sB, cB)):
                    nc.vector.tensor_reduce(
                        out=sh[:],
                        in_=vgh[:, :, :, 0:K].rearrange(
                            "p f t k -> p t k f"),
                        axis=Ax.X,
                        op=Alu.add,
                    )
                    nc.vector.tensor_reduce(
                        out=ch[:],
                        in_=vgh[:, :, :, K].rearrange("p f t -> p t f"),
                        axis=Ax.X,
                        op=Alu.add,
                    )
                s_all = pool.tile([P, NTILES, K], f32, tag="s")
                nc.vector.tensor_tensor(
                    out=s_all[:], in0=sA[:], in1=sB[:], op=Alu.add
                )
                csum = pool.tile([P, NTILES], f32, tag="c")
                nc.vector.tensor_tensor(
                    out=csum[:], in0=cA[:], in1=cB[:], op=Alu.add
                )
                # table stores vred*sqrt(0.5), so |s|^2 here IS the
                # 0.5*|sum vred|^2 term -- no scalar-mul needed
                ssq = pool.tile([P, NTILES, K], f32, tag="ssq")
                nc.vector.tensor_tensor(
                    out=ssq[:], in0=s_all[:], in1=s_all[:], op=Alu.mult
                )
                s2 = pool.tile([P, NTILES], f32, tag="s2")
                nc.vector.tensor_reduce(
                    out=s2[:], in_=ssq[:], axis=Ax.X, op=Alu.add
                )
                out_all = pool.tile([P, NTILES], f32, tag="oa")
                nc.vector.tensor_tensor(
                    out=out_all[:], in0=s2[:], in1=csum[:], op=Alu.add
                )
                # single store: out[t*128+p] = out_all[p, t]
                nc.sync.dma_start(
                    out=out_d[:, :].rearrange("(t p) one -> p (t one)", p=P),
                    in_=out_all[:],
                )
    nc.compile()
    return nc


def get_nc():
    if "nc" not in _cache:
        _cache["nc"] = _build_nc()
    return _cache["nc"]


def make_in_maps(inputs, offsets, w0, w, v):
    del offsets  # folded into the per-field subtable slicing
    inp = np.asarray(inputs)
    idx16 = np.ascontiguousarray(
        inp.astype(np.int16).reshape(NCORES, BC, FIELD)
    )
    # reduced table row: [vred (8 f32) | c | pad to 64 f32 = 256 B]
    vred = np.asarray(v, dtype=np.float32).reshape(TOTAL, FIELD, K).sum(axis=1)
    c = (np.asarray(w, dtype=np.float32).reshape(TOTAL)
         + np.float32(np.asarray(w0, np.float32).reshape(()) / FIELD)
         - 0.5 * (vred * vred).sum(axis=1))
    tab = np.zeros((TOTAL, RPAD), dtype=np.float32)
    # sqrt(0.5) fold: the device's |sum_f row|^2 then equals 0.5*|s|^2
    tab[:, :K] = vred * np.float32(np.sqrt(0.5))
    tab[:, K] = c
    maps = []
    for i in range(NCORES):
        shard = idx16[i]                       # [BC, FIELD]
        wrapped = shard.reshape(NSLOT, 16, FIELD).transpose(1, 2, 0)
        # [16, FIELD, NSLOT] -> replicate to 128 partitions
        rep = np.ascontiguousarray(np.tile(wrapped, (NCORES, 1, 1)))
        maps.append({"idx16": rep, "tab": tab})
    return maps


def assemble_out(res):
    return np.concatenate(
        [np.asarray(res.results[i]["out"]) for i in range(NCORES)], axis=0
    ).astype(np.float32)


def kernel(inputs, offsets, w0, w, v):
    if _TRN_REPO not in sys.path:
        sys.path.insert(0, _TRN_REPO)
    from concourse.bass_utils import run_bass_kernel_spmd

    nc = get_nc()
    in_maps = make_in_maps(inputs, offsets, w0, w, v)
    res = run_bass_kernel_spmd(nc, in_maps, list(range(NCORES)))
    return assemble_out(res)
